# revision 9
# baseline (speedup 1.0000x reference)
"""AffinityLoss BCE kernel for 8 Trainium2 NeuronCores.

Computes mean BCE between prediction [4,4096,4096] (probabilities) and the
pairwise label-equality affinity derived from target [4,512,512]:

    aff[b,i,j] = (lab[b,i] == lab[b,j]),  lab = target[:, ::8, ::8].flatten
    loss = mean( -(aff*log(p) + (1-aff)*log(1-p)) )

Per-element identity (one transcendental per element):
    -loss_elem = log(q),  q = p if aff else (1-p)

Key trick: the host permutes the j-columns of each batch by label order
(sum is permutation-invariant), which turns each row's affinity mask into
one contiguous index range [s_i, e_i).  A custom DVE op then computes

    q = select(s <= Idx < e, p, 1-p)        # one single-read Vector pass

with per-partition range scalars, and ScalarE does Ln(q) with accum_out.
No mask tensors, no tensor_tensor pass, no matmuls.

Sharding: data-parallel over rows; core c handles batch c//2, row half
c%2 (2048 rows = 16 blocks of 128).  Each core returns per-(partition,
pair) partial sums; the host sums in float64 and divides by the count.
"""

import numpy as np

import concourse.bacc as bacc
import concourse.tile as tile
import concourse.mybir as mybir
from concourse import bass_utils
from concourse import dve_ops
from concourse.dve_spec import Spec, Src0, C0, C1, Idx, One, select, lower, _has_src1
from concourse.dve_uop import DveOpSpec

B = 4
N = 4096            # (512//8)**2
STRIDE = 8
NUM_CLASSES = 182
IGNORE = 255
N_CORES = 8
ROWS_PER_CORE = (B * N) // N_CORES   # 2048
P = 128
BLOCKS = ROWS_PER_CORE // P          # 16
PAIRS = BLOCKS // 2                  # 8: two row-blocks per compute pass
F = N                                # free dim of one block

_AFF_NAME = "AFFINITY_RANGE_Q_ANT"
_cache = {}
last_results = None  # test harness reads exec_time_ns off this


def _aff_ref(in0, in1, c0, c1, c2):
    x = np.asarray(in0, dtype=np.float32)
    x2 = x.reshape(x.shape[0], -1)
    idx = np.arange(x2.shape[1], dtype=np.float32)[None, :]
    s = np.asarray(c0, dtype=np.float32).reshape(-1, 1)
    e = np.asarray(c1, dtype=np.float32).reshape(-1, 1)
    out = np.where((idx >= s) & (idx < e), x2, np.float32(1.0) - x2)
    return out.reshape(x.shape).astype(np.float32)


def _register_aff_op():
    for op in dve_ops.OPS:
        if op.name == _AFF_NAME:
            return op
    body = select((Idx >= C0) & (Idx < C1), Src0, One - Src0)
    spec = Spec(body=body, reference=_aff_ref)
    row = max(dve_ops._SUB_OPCODE_FOR_NAME.values()) + 1
    assert row < 0x20
    rd1 = _has_src1(spec)
    shas = {}
    for ver in ("v3", "v4"):
        try:
            s = DveOpSpec(name=_AFF_NAME, opcode=row, uops=lower(spec, ver=ver),
                          rd1_en=rd1)
            shas[ver] = s.sha(ver)
        except Exception:
            pass
    op = dve_ops.DveOp(_AFF_NAME, spec, subdim=False, uops_sha=shas)
    dve_ops.OPS.append(op)
    dve_ops.CUSTOM_DVE_SPECS[_AFF_NAME] = spec
    dve_ops._SUB_OPCODE_FOR_NAME[_AFF_NAME] = row
    return op


def _build():
    if "nc" in _cache:
        return _cache["nc"]

    aff_op = _register_aff_op()

    f32 = mybir.dt.float32
    Act = mybir.ActivationFunctionType

    nc = bacc.Bacc("TRN2", target_bir_lowering=False, debug=False)
    pred = nc.dram_tensor("pred", [ROWS_PER_CORE, F], f32, kind="ExternalInput").ap()
    ms = nc.dram_tensor("ms", [P, BLOCKS], f32, kind="ExternalInput").ap()
    me = nc.dram_tensor("me", [P, BLOCKS], f32, kind="ExternalInput").ap()
    acc = nc.dram_tensor("acc", [P, PAIRS], f32, kind="ExternalOutput").ap()

    with tile.TileContext(nc) as tc:
        with (
            tc.tile_pool(name="const", bufs=1) as cpool,
            tc.tile_pool(name="pin", bufs=3) as ppool,
        ):
            ms_sb = cpool.tile([P, BLOCKS], f32, tag="ms")
            nc.sync.dma_start(ms_sb[:], ms[:])
            me_sb = cpool.tile([P, BLOCKS], f32, tag="me")
            nc.sync.dma_start(me_sb[:], me[:])
            acc_sb = cpool.tile([P, PAIRS], f32, tag="acc")

            for u in range(PAIRS):
                t0, t1 = 2 * u, 2 * u + 1
                # two row-blocks side by side in the free dim; one 4 MiB DMA
                # per block, split across the two HWDGE rings
                p_t = ppool.tile([P, 2 * F], f32, tag="p")
                nc.sync.dma_start(p_t[:, :F], pred[t0 * P:(t0 + 1) * P, :])
                nc.scalar.dma_start(p_t[:, F:], pred[t1 * P:(t1 + 1) * P, :])

                # in-place per block: p = (s <= j < e) ? p : 1-p   (= q)
                for k, t in ((0, t0), (1, t1)):
                    nc.vector._custom_dve(
                        aff_op,
                        out=p_t[:, k * F:(k + 1) * F],
                        in0=p_t[:, k * F:(k + 1) * F],
                        s0=ms_sb[:, t:t + 1],
                        s1=me_sb[:, t:t + 1],
                    )
                # in-place: p = Ln(q); acc col = row-sum
                nc.scalar.activation(
                    p_t[:], p_t[:], Act.Ln, accum_out=acc_sb[:, u:u + 1],
                )

            nc.sync.dma_start(acc[:], acc_sb[:])

    nc.compile()
    _cache["nc"] = nc
    return nc


def make_in_maps(prediction, target):
    prediction = np.asarray(prediction, dtype=np.float32)
    target = np.asarray(target)
    lab = target[:, ::STRIDE, ::STRIDE]
    lab = np.where(lab == IGNORE, NUM_CLASSES, lab)
    flat = lab.reshape(B, N).astype(np.int64)

    in_maps = []
    per_batch = N_CORES // B
    for b in range(B):
        labs = flat[b]
        perm = np.argsort(labs, kind="stable")          # column order by label
        cum = np.zeros(NUM_CLASSES + 2, dtype=np.int64)
        np.cumsum(np.bincount(labs, minlength=NUM_CLASSES + 1), out=cum[1:])
        pred_perm = prediction[b][:, perm]              # [4096, 4096]
        starts = cum[labs].astype(np.float32)           # [4096] per-row range
        ends = cum[labs + 1].astype(np.float32)
        for h in range(per_batch):
            r0 = h * ROWS_PER_CORE
            rows = slice(r0, r0 + ROWS_PER_CORE)
            in_maps.append({
                "pred": np.ascontiguousarray(pred_perm[rows]),
                "ms": np.ascontiguousarray(
                    starts[rows].reshape(BLOCKS, P).T),
                "me": np.ascontiguousarray(
                    ends[rows].reshape(BLOCKS, P).T),
            })
    return in_maps


def kernel(prediction, target):
    global last_results
    nc = _build()
    in_maps = make_in_maps(prediction, target)
    res = bass_utils.run_bass_kernel_spmd(nc, in_maps, core_ids=list(range(N_CORES)))
    last_results = res
    total = 0.0
    for r in res.results:
        total += r["acc"].astype(np.float64).sum()
    loss = -total / float(B * N * N)
    return np.float32(loss)
